# revision 8
# baseline (speedup 1.0000x reference)
"""MAMGCN submodule kernel for Trainium2, 8-core data-parallel over batch.

Problem (per reference):
  B=16, N=1024, F=64, T=12, K=3, F_OUT=64
  S = softmax_axis1(Vs @ sigmoid(lhs @ rhs^T + bs))
  out = relu(sum_k (cheb_k * S)^T @ x @ Theta_k)

Sharding: batch B=16 split across 8 cores (2 batches/core). All weights
replicated. Each core runs an identical Bass program on its shard.

v2 layout strategy (vs the prior all-PE baseline, 259us/iter):
  - All PE transposes removed: x' -> xT chunk transposes and the output
    (t,o)->n transposes both run on the DMA XBAR (dma_start_transpose,
    one instruction per m-tile / per (q,half)), freeing ~21us/iter of
    PE issue time and the LDWEIGHTS(transpose) slots.
  - The W1/W3 row features (xw1 = sum_t x*W1, rhs = sum_f W3*x) are no
    longer DVE reduce pipelines: both come out of one accumulated PE
    matmul per n-half over the xT chunks with a combined [128,76]
    stationary (64 cols of W1-scaled identity + 12 cols of W3 columns),
    producing xw1T and rhsBT directly K-partitioned -- no transposes,
    no DVE reduces, no gpsimd muls.
  - A_k = cheb_k * E muls split DVE (k=0,1) / gpsimd (k=2) so the DVE
    never backs up the z' tensor queue (the old 2.1-3.4us S[156] stalls).
  - bs cached in SBUF as bf16 at setup: stage C adds are 2x-rate DVE
    ops with no per-batch bs DMA.
  - Everything else as before: bf16 matmuls with fp32 PSUM, n in
    512-wide halves, z' with x'(t,f)-chunk stationaries, Theta via
    block-diagonal stationary, softmax denominator + relu folded into
    one tensor_scalar per 128-block, next batch's stage A drained as
    thunks into this batch's C/D/F emission.
"""
import numpy as np

import concourse.bass as bass
import concourse.mybir as mybir
import concourse.tile as tile
from concourse import bacc
from concourse.bass_utils import run_bass_kernel_spmd
from concourse.masks import make_identity

F32 = mybir.dt.float32
F32R = mybir.dt.float32r
BF16 = mybir.dt.bfloat16
AL = mybir.AluOpType
AF = mybir.ActivationFunctionType
AX = mybir.AxisListType

B_PER_CORE = 2
N = 1024
F = 64
T = 12
K = 3
FO = 64
NT = N // 128           # 8 n-tiles (128 rows each)
NH = 2                  # n processed in halves
HW = N // NH            # 512 free-dim per half
TQ = (T * F) // 128     # 6 (t,f)-chunks (each = 2 t-values x 64 f)


def _make_stage_a(nc, pools, cst, b, x_d):
    """Allocate stage-A tiles for batch b and return (thunks, tiles).

    Stage A is now just: DMA x in, cast/reorder to x' (m-part, bf16),
    and one XBAR DMA-transpose per m-tile into the (t,f)-partitioned
    xT chunks. Cut into per-m-tile thunks the caller interleaves into
    the PREVIOUS batch's C/D/F emission."""
    (stream, bigp, pe_pool, res_pool, psZ, psG, dram_pool) = pools
    xprime = bigp.tile([128, NT, T, F], BF16, tag="xp", bufs=2, name="xprime")
    xTall = bigp.tile([128, TQ, N], BF16, tag="xT", bufs=1, name="xTall")
    thunks = []
    for mi in range(NT):
        MS = slice(mi * 128, (mi + 1) * 128)
        xnat = stream.tile([128, F, T], F32, tag="xnat", bufs=2, name="xnat")

        def t_dma(xnat=xnat, MS=MS):
            nc.gpsimd.dma_start(out=xnat[:], in_=x_d.ap()[b, MS])

        def t_xp(xnat=xnat, mi=mi):
            nc.vector.tensor_copy(xprime[:, mi],
                                  xnat[:].rearrange("p f t -> p t f"))

        def t_tr(mi=mi, MS=MS):
            # xTall[p=(dt,f), q, m] = xprime[m, t=2q+dt, f]
            nc.sync.dma_start_transpose(
                out=xTall[:, :, MS],
                in_=xprime[:, mi].rearrange("p t f -> p (t f)"))

        thunks += [t_dma, t_xp, t_tr]
    return thunks, (xprime, xTall)


def _emit_batch(nc, tc, pools, cst, b, x_d, out_d, a_tiles, next_thunks):
    """Emit one batch's B/C/D/F pipeline, draining next batch's stage-A
    thunks at fixed points along the way."""
    (stream, bigp, pe_pool, res_pool, psZ, psG, dram_pool) = pools
    xprime, xTall = a_tiles

    def drain(n=1):
        for _ in range(n):
            if next_thunks:
                next_thunks.pop(0)()

    # ---- Stage W: xw1T (64, N) and rhsBT (12, N) via accumulated PE
    # matmuls over the xT chunks with the combined W1/W3 stationary ----
    xw1T = stream.tile([F, N], BF16, tag="xw1T", bufs=2, name="xw1T")
    rhsBT = stream.tile([T, N], BF16, tag="rhsBT", bufs=2, name="rhsBT")
    for h in range(NH):
        HS = slice(h * HW, (h + 1) * HW)
        ps_w = psG.tile([F + T, HW], F32, tag="g", name="ps_w")
        for q in range(TQ):
            nc.tensor.matmul(ps_w[:], cst["wc"][:, q, :], xTall[:, q, HS],
                             start=(q == 0), stop=(q == TQ - 1))
        nc.scalar.copy(xw1T[:, HS], ps_w[0:F])
        nc.scalar.copy(rhsBT[:, HS], ps_w[F:F + T])

    # ---- Stage B: lhsT = W2^T @ xw1T  (12, N) ----
    # psum->sbuf copy on ACT: keeps stage C's LDWEIGHTS dependency off
    # the (potentially backed-up) DVE FIFO at batch boundaries.
    lhsT_sb = stream.tile([T, N], BF16, tag="lhsT", bufs=1)
    for h in range(2):
        ps_l = psG.tile([T, 512], F32, tag="g")
        nc.tensor.matmul(ps_l[:], cst["w2r"][:], xw1T[:, h * 512:(h + 1) * 512],
                         start=True, stop=True)
        nc.scalar.copy(lhsT_sb[:, h * 512:(h + 1) * 512], ps_l[:])

    # ---- per n-half pipeline ----
    for nh in range(NH):
        HS = slice(nh * HW, (nh + 1) * HW)
        # Stage C: product + bs -> sigmoid -> P  (rows = m, cols = dest n)
        P_h = pe_pool.tile([128, NT, HW], BF16, tag="P", bufs=1)
        for mi in range(NT):
            MS = slice(mi * 128, (mi + 1) * 128)
            ps_p = psG.tile([128, HW], F32, tag="g")
            nc.tensor.matmul(ps_p[:], lhsT_sb[:, MS], rhsBT[:, HS],
                             start=True, stop=True)
            sgin = stream.tile([128, HW], BF16, tag="sgin", bufs=3)
            nc.vector.tensor_add(sgin[:], ps_p[:], cst["bs_sb"][:, mi, HS])
            nc.scalar.activation(P_h[:, mi], sgin[:], AF.Sigmoid)
            drain(1)
        # Stage D: S = VsT^T @ P ; E = exp(S).  A_k = cheb_k * E muls run on
        # DVE (k=0,1) and gpsimd (k=2) behind the PE; colsum matmuls are
        # deferred past the ii loop so the tensor queue never waits on exp.
        E_h = pe_pool.tile([128, NT, HW], BF16, tag="E", bufs=1)
        A_t = [pe_pool.tile([128, NT, HW], BF16, tag=f"A{k}", bufs=1,
                            name=f"A{k}") for k in range(K)]
        for ii in range(NT):
            ps_s = psG.tile([128, HW], F32, tag="g")
            for pi in range(NT):
                nc.tensor.matmul(ps_s[:], cst["vsT"][:, pi, ii * 128:(ii + 1) * 128],
                                 P_h[:, pi], start=(pi == 0), stop=(pi == NT - 1))
            nc.scalar.activation(E_h[:, ii], ps_s[:], AF.Exp)
            nc.vector.tensor_mul(A_t[0][:, ii], cst["chebb"][:, 0, ii, HS],
                                 E_h[:, ii])
            nc.vector.tensor_mul(A_t[1][:, ii], cst["chebb"][:, 1, ii, HS],
                                 E_h[:, ii])
            nc.gpsimd.tensor_mul(A_t[2][:, ii], cst["chebb"][:, 2, ii, HS],
                                 E_h[:, ii])
            drain(1)
        ps_cs = psG.tile([1, HW], F32, tag="g")
        for ii in range(NT):
            nc.tensor.matmul(ps_cs[:], cst["ones_b"][:], E_h[:, ii],
                             start=(ii == 0), stop=(ii == NT - 1))
        # softmax denominator reciprocal, scattered to partitions via DRAM
        cs_row = stream.tile([1, HW], F32, tag="cs", bufs=1)
        nc.vector.tensor_copy(cs_row[:], ps_cs[:])
        rc_d = dram_pool.tile([HW], F32, tag="rcd", name="rc_d")
        nc.sync.dma_start(out=rc_d.rearrange("(a b) -> a b", a=1), in_=cs_row[:])
        rc_sc = stream.tile([128, HW // 128], F32, tag="rcsc", bufs=2)
        nc.sync.dma_start(out=rc_sc[:], in_=rc_d.rearrange("(c p) -> p c", p=128))
        recip_sb = stream.tile([128, HW // 128], F32, tag="recip", bufs=2)
        nc.vector.reciprocal(recip_sb[:], rc_sc[:])
        # Stage F: z'_k = x'-chunk^T @ A_k ; Theta via block-diag; output
        # transpose on the DMA XBAR; relu/softmax-scale per 128-block.
        res_tiles = []
        for _c in range(HW // 128):
            res_c = res_pool.tile([128, FO, T], F32, tag=f"res{_c}", bufs=1,
                                  name=f"res{_c}")
            res_tiles.append(res_c)

        def _emit_theta(zs_tiles, q):
            """Transpose-fused Theta for chunk q: the 128-col n-slices of
            zs_k act as the stationary and thbd_k streams through, so the
            output lands n-partitioned in PSUM (no transpose anywhere).
            Emitted one q behind the z' matmuls so the tensor queue never
            waits on the ACT psum->sbuf copies."""
            ps_t = psG.tile([128, HW // 128, 128], F32, tag="g", name="ps_t")
            for ns in range(HW // 128):
                for k in range(K):
                    nc.tensor.matmul(ps_t[:, ns],
                                     zs_tiles[k][:, ns * 128:(ns + 1) * 128],
                                     cst["thbd"][:, k, :],
                                     start=(k == 0), stop=(k == K - 1))
                nc.vector.tensor_scalar(
                    out=res_tiles[ns][:, :, 2 * q:2 * q + 2],
                    in0=ps_t[:, ns].rearrange("p (dt o) -> p o dt", o=FO),
                    scalar1=recip_sb[:, ns:ns + 1],
                    scalar2=0.0,
                    op0=AL.mult,
                    op1=AL.max,
                )

        pend = None
        for q in range(TQ):
            zs_tiles = []
            for k in range(K):
                ps_z = psZ.tile([128, HW], F32, tag="z")
                for mi in range(NT):
                    nc.tensor.matmul(ps_z[:], xprime[:, mi, 2 * q:2 * q + 2, :],
                                     A_t[k][:, mi],
                                     start=(mi == 0), stop=(mi == NT - 1))
                zs = stream.tile([128, HW], BF16, tag="zs", bufs=6)
                nc.scalar.copy(zs[:], ps_z[:])
                zs_tiles.append(zs)
            if pend is not None:
                _emit_theta(*pend)
            pend = (zs_tiles, q)
            drain(2)
        _emit_theta(*pend)
        for c in range(HW // 128):
            nt_i = nh * (HW // 128) + c
            nc.sync.dma_start(
                out=out_d.ap()[b, nt_i * 128:(nt_i + 1) * 128],
                in_=res_tiles[c][:])
    drain(len(next_thunks))


def build_nc(repeat=1):
    nc = bacc.Bacc("TRN2", target_bir_lowering=False, debug=False, num_devices=8)
    x_d = nc.dram_tensor("x", [B_PER_CORE, N, F, T], F32, kind="ExternalInput")
    w1_d = nc.dram_tensor("W1", [T], F32, kind="ExternalInput")
    w2_d = nc.dram_tensor("W2", [F, T], F32, kind="ExternalInput")
    w3_d = nc.dram_tensor("W3", [F], F32, kind="ExternalInput")
    bs_d = nc.dram_tensor("bs", [N, N], F32, kind="ExternalInput")
    vs_d = nc.dram_tensor("Vs", [N, N], F32, kind="ExternalInput")
    cheb_d = nc.dram_tensor("cheb", [K, N, N], F32, kind="ExternalInput")
    th_d = nc.dram_tensor("Theta", [K, F, FO], F32, kind="ExternalInput")
    out_d = nc.dram_tensor("out", [B_PER_CORE, N, FO, T], F32,
                           kind="ExternalOutput")

    with tile.TileContext(nc) as tc:
        with (
            tc.tile_pool(name="consts", bufs=1) as consts,
            tc.tile_pool(name="stream", bufs=2) as stream,
            tc.tile_pool(name="bigp", bufs=1) as bigp,
            tc.tile_pool(name="pe", bufs=1) as pe_pool,
            tc.tile_pool(name="res", bufs=1) as res_pool,
            tc.tile_pool(name="dram", bufs=2, space="DRAM") as dram_pool,
            tc.tile_pool(name="psZ", bufs=2, space="PSUM") as psZ,
            tc.tile_pool(name="psG", bufs=6, space="PSUM") as psG,
        ):
            cst = {}
            identb = consts.tile([128, 128], BF16)
            make_identity(nc, identb[:])
            cst["identb"] = identb
            # ones vector (bf16) for the softmax column sums
            onesf = consts.tile([128, 1], F32)
            nc.vector.memset(onesf[:], 1.0)
            ones_b = consts.tile([128, 1], BF16)
            nc.vector.tensor_copy(ones_b[:], onesf[:])
            cst["ones_b"] = ones_b
            # ---- combined W1/W3 stationary WC [128, TQ, 76]:
            #   cols 0:64   -> xw1T rows:  WC[(dt,f), q, f'] = W1[2q+dt]*d(f==f')
            #   cols 64:76  -> rhsBT rows: WC[(dt,f), q, 64+t'] = W3[f]*d(t'==2q+dt)
            # d2[p, j] = d(p % 64 == j)  (two stacked 64-identities)
            d2 = consts.tile([128, F], BF16)
            nc.vector.tensor_copy(d2[0:F], identb[0:F, 0:F])
            nc.vector.tensor_copy(d2[F:128], identb[F:128, F:128])
            # W1 broadcast to all partitions: wcol[p, t] = W1[t]
            wcol = stream.tile([128, T], F32, tag="xnat", bufs=2, name="wcol")
            nc.gpsimd.dma_start(
                out=wcol[:],
                in_=bass.AP(tensor=w1_d, offset=0, ap=[[0, 128], [1, T]]))
            # w1col[p, q] = W1[2q + p//64]
            w1col = consts.tile([128, TQ], F32)
            for q in range(TQ):
                nc.vector.tensor_copy(w1col[0:F, q:q + 1],
                                      wcol[0:F, 2 * q:2 * q + 1])
                nc.vector.tensor_copy(w1col[F:128, q:q + 1],
                                      wcol[F:128, 2 * q + 1:2 * q + 2])
            # W3 broadcast, then w3col[p] = W3[p % 64] via d2 reduce
            wst2 = stream.tile([128, F], F32, tag="xnat", bufs=2, name="wst2")
            nc.gpsimd.dma_start(
                out=wst2[:],
                in_=bass.AP(tensor=w3_d, offset=0, ap=[[0, 128], [1, F]]))
            tmp64 = stream.tile([128, F], F32, tag="xnat", bufs=2, name="tmp64")
            nc.vector.tensor_mul(tmp64[:], d2[:], wst2[:])
            w3col = consts.tile([128, 1], F32)
            nc.vector.tensor_reduce(out=w3col[:], in_=tmp64[:], op=AL.add,
                                    axis=AX.X)
            wc = consts.tile([128, TQ, F + T], BF16)
            nc.vector.memset(wc[:], 0.0)
            for q in range(TQ):
                nc.vector.tensor_scalar(
                    out=wc[:, q, 0:F], in0=d2[:],
                    scalar1=w1col[:, q:q + 1], scalar2=0.0,
                    op0=AL.mult, op1=AL.add)
                nc.vector.tensor_copy(wc[0:F, q, F + 2 * q:F + 2 * q + 1],
                                      w3col[0:F])
                nc.vector.tensor_copy(wc[F:128, q, F + 2 * q + 1:F + 2 * q + 2],
                                      w3col[F:128])
            cst["wc"] = wc
            # W2 (f, t) bf16 stationary
            w2f = consts.tile([F, T], F32)
            nc.sync.dma_start(out=w2f[:], in_=w2_d.ap())
            w2r = consts.tile([F, T], BF16)
            nc.vector.tensor_copy(w2r[:], w2f[:])
            cst["w2r"] = w2r
            # block-diagonal Theta (128, K, 128), bf16 stationary
            thbd_f = consts.tile([128, K, 128], F32)
            nc.vector.memset(thbd_f[:], 0.0)
            for k in range(K):
                nc.sync.dma_start(out=thbd_f[0:F, k, 0:FO], in_=th_d.ap()[k])
                nc.sync.dma_start(out=thbd_f[F:128, k, FO:128], in_=th_d.ap()[k])
            thbd = consts.tile([128, K, 128], BF16)
            nc.vector.tensor_copy(thbd[:], thbd_f[:])
            cst["thbd"] = thbd
            # cheb cached in SBUF as bf16 (one-time cast) so the A_k muls
            # run at the 16-bit 2x DVE rate with no per-batch cheb DMA
            chebb = consts.tile([128, K, NT, N], BF16)
            for k in range(K):
                for mi in range(NT):
                    chst = stream.tile([128, N], F32, tag="xnat", bufs=2, name="chst")
                    nc.gpsimd.dma_start(
                        out=chst[:],
                        in_=cheb_d.ap()[k, mi * 128:(mi + 1) * 128])
                    nc.vector.tensor_copy(chebb[:, k, mi], chst[:])
            cst["chebb"] = chebb
            # bs cached in SBUF as bf16 (one-time cast)
            bs_sb = consts.tile([128, NT, N], BF16)
            for mi in range(NT):
                bst = stream.tile([128, N], F32, tag="xnat", bufs=2, name="bst")
                nc.gpsimd.dma_start(
                    out=bst[:], in_=bs_d.ap()[mi * 128:(mi + 1) * 128])
                nc.vector.tensor_copy(bs_sb[:, mi], bst[:])
            cst["bs_sb"] = bs_sb
            # VsT (m-partitioned Vs transpose), bf16 stationary
            vsT = consts.tile([128, NT, N], BF16)
            identf = consts.tile([128, 128], F32)
            make_identity(nc, identf[:])
            for pi in range(NT):
                for ii in range(NT):
                    vtmp = stream.tile([128, 128], F32, tag="xnat", bufs=2, name="vtmp")
                    nc.sync.dma_start(
                        out=vtmp[:],
                        in_=vs_d.ap()[ii * 128:(ii + 1) * 128,
                                      pi * 128:(pi + 1) * 128])
                    ps_v = psG.tile([128, 128], F32, tag="g")
                    nc.tensor.transpose(ps_v[:], vtmp[:], identf[:])
                    nc.vector.tensor_copy(vsT[:, pi, ii * 128:(ii + 1) * 128],
                                          ps_v[:])
            cst["vsT"] = vsT

            pools = (stream, bigp, pe_pool, res_pool, psZ, psG, dram_pool)
            # software pipeline: batch b+1's stage A interleaves into batch
            # b's C/D/F emission; the first batch's stage A runs up front.
            a_thunks, a_tiles = _make_stage_a(nc, pools, cst, 0, x_d)
            for t in a_thunks:
                t()
            total = repeat * B_PER_CORE
            for i in range(total):
                b = i % B_PER_CORE
                if i + 1 < total:
                    n_thunks, n_tiles = _make_stage_a(nc, pools, cst,
                                                      (i + 1) % B_PER_CORE, x_d)
                else:
                    n_thunks, n_tiles = [], None
                _emit_batch(nc, tc, pools, cst, b, x_d, out_d,
                            a_tiles, n_thunks)
                a_tiles = n_tiles
    nc.compile()
    return nc


_RUNNER_CACHE = {}


def _make_runner(repeat=1):
    """Build the Bass program once and wrap it in a persistent jitted
    shard_map executable so repeat calls skip recompile/reload."""
    import jax
    from jax.sharding import Mesh, PartitionSpec
    from jax.experimental.shard_map import shard_map
    from concourse import bass2jax, mybir as _mybir

    nc = build_nc(repeat)
    bass2jax.install_neuronx_cc_hook()

    part_name = nc.partition_id_tensor.name if nc.partition_id_tensor else None
    in_names = []
    out_names = []
    out_avals = []
    zero_outs = []
    for alloc in nc.m.functions[0].allocations:
        if not isinstance(_mybir.MemoryLocationSet, type) or not isinstance(
                alloc, _mybir.MemoryLocationSet):
            continue
        name = alloc.memorylocations[0].name
        if alloc.kind == "ExternalInput":
            if name != part_name:
                in_names.append(name)
        elif alloc.kind == "ExternalOutput":
            out_names.append(name)
            shape = tuple(alloc.tensor_shape)
            dtype = _mybir.dt.np(alloc.dtype)
            out_avals.append(jax.core.ShapedArray(shape, dtype))
            zero_outs.append(np.zeros(shape, dtype))
    n_params = len(in_names)
    all_names = in_names + out_names
    if part_name is not None:
        all_names = all_names + [part_name]

    def _body(*args):
        operands = list(args)
        if part_name is not None:
            operands.append(bass2jax.partition_id_tensor())
        outs = bass2jax._bass_exec_p.bind(
            *operands,
            out_avals=tuple(out_avals),
            in_names=tuple(all_names),
            out_names=tuple(out_names),
            lowering_input_output_aliases=(),
            sim_require_finite=False,
            sim_require_nnan=False,
            nc=nc,
        )
        return tuple(outs)

    n_cores = 8
    devices = jax.devices()[:n_cores]
    mesh = Mesh(np.asarray(devices), ("core",))
    in_specs = tuple(
        PartitionSpec("core") if name == "x" else PartitionSpec()
        for name in in_names
    ) + (PartitionSpec("core"),) * len(out_names)
    out_specs = (PartitionSpec("core"),) * len(out_names)
    sharded = jax.jit(
        shard_map(_body, mesh=mesh, in_specs=in_specs, out_specs=out_specs,
                  check_rep=False),
        keep_unused=True,
    )
    return nc, sharded, in_names, out_names, zero_outs, n_cores, mesh


def _get_runner(repeat=1):
    if repeat not in _RUNNER_CACHE:
        _RUNNER_CACHE[repeat] = _make_runner(repeat)
    return _RUNNER_CACHE[repeat]


def kernel(x, W1, W2, W3, bs, Vs, cheb, Theta, repeat=1):
    x = np.asarray(x, dtype=np.float32)
    full = {
        "W1": np.asarray(W1, dtype=np.float32),
        "W2": np.asarray(W2, dtype=np.float32),
        "W3": np.asarray(W3, dtype=np.float32),
        "bs": np.asarray(bs, dtype=np.float32).reshape(N, N),
        "Vs": np.asarray(Vs, dtype=np.float32),
        "cheb": np.asarray(cheb, dtype=np.float32),
        "Theta": np.asarray(Theta, dtype=np.float32),
    }
    nc, sharded, in_names, out_names, zero_outs, n_cores, mesh = _get_runner(repeat)
    ops = _staged_ops(x, full, in_names, zero_outs, n_cores)
    out_arrs = sharded(*ops)
    out = np.asarray(out_arrs[out_names.index("out")])
    return out.reshape(16, N, FO, T)


def _staged_ops(x, full, in_names, zero_outs, n_cores):
    ops = []
    for name in in_names:
        if name == "x":
            ops.append(np.ascontiguousarray(x.reshape(n_cores * B_PER_CORE, N, F, T)))
        else:
            ops.append(full[name])
    for z in zero_outs:
        ops.append(np.zeros((n_cores * z.shape[0], *z.shape[1:]), z.dtype))
    return ops


def _bench_setup(inputs, repeat):
    import jax
    from jax.sharding import NamedSharding, PartitionSpec
    x = np.asarray(inputs["x"], dtype=np.float32)
    full = {k: np.asarray(v, dtype=np.float32) for k, v in inputs.items() if k != "x"}
    full["bs"] = full["bs"].reshape(N, N)
    nc, sharded, in_names, out_names, zero_outs, n_cores, mesh = _get_runner(repeat)
    ops = _staged_ops(x, full, in_names, zero_outs, n_cores)
    sh_core = NamedSharding(mesh, PartitionSpec("core"))
    sh_rep = NamedSharding(mesh, PartitionSpec())
    shardings = [sh_core if name == "x" else sh_rep for name in in_names]
    shardings += [sh_core] * len(zero_outs)
    dev_ops = [jax.device_put(o, s_) for o, s_ in zip(ops, shardings)]
    jax.block_until_ready(sharded(*dev_ops))
    return sharded, dev_ops


def bench_pair(inputs, rep_a=1, rep_b=9, iters=20):
    """Interleaved device-resident timing of two repeat variants.
    Returns (best_a, best_b) seconds — interleaving cancels slow drift in the
    fixed dispatch overhead."""
    import time as _time
    import jax
    sh_a, ops_a = _bench_setup(inputs, rep_a)
    sh_b, ops_b = _bench_setup(inputs, rep_b)
    best_a = best_b = float("inf")
    for _ in range(iters):
        t0 = _time.time()
        jax.block_until_ready(sh_a(*ops_a))
        best_a = min(best_a, _time.time() - t0)
        t0 = _time.time()
        jax.block_until_ready(sh_b(*ops_b))
        best_b = min(best_b, _time.time() - t0)
    return best_a, best_b


# revision 13
# speedup vs baseline: 1.0828x; 1.0828x over previous
"""MAMGCN submodule kernel for Trainium2, 8-core data-parallel over batch.

Problem (per reference):
  B=16, N=1024, F=64, T=12, K=3, F_OUT=64
  S = softmax_axis1(Vs @ sigmoid(lhs @ rhs^T + bs))
  out = relu(sum_k (cheb_k * S)^T @ x @ Theta_k)

Sharding: batch B=16 split across 8 cores (2 batches/core). All weights
replicated. Each core runs an identical Bass program on its shard.

v2 layout strategy (vs the prior all-PE baseline, 259us/iter):
  - All PE transposes removed: x' -> xT chunk transposes and the output
    (t,o)->n transposes both run on the DMA XBAR (dma_start_transpose,
    one instruction per m-tile / per (q,half)), freeing ~21us/iter of
    PE issue time and the LDWEIGHTS(transpose) slots.
  - The W1/W3 row features (xw1 = sum_t x*W1, rhs = sum_f W3*x) are no
    longer DVE reduce pipelines: both come out of one accumulated PE
    matmul per n-half over the xT chunks with a combined [128,76]
    stationary (64 cols of W1-scaled identity + 12 cols of W3 columns),
    producing xw1T and rhsBT directly K-partitioned -- no transposes,
    no DVE reduces, no gpsimd muls.
  - A_k = cheb_k * E muls split DVE (k=0,1) / gpsimd (k=2) so the DVE
    never backs up the z' tensor queue (the old 2.1-3.4us S[156] stalls).
  - bs cached in SBUF as bf16 at setup: stage C adds are 2x-rate DVE
    ops with no per-batch bs DMA.
  - Everything else as before: bf16 matmuls with fp32 PSUM, n in
    512-wide halves, z' with x'(t,f)-chunk stationaries, Theta via
    block-diagonal stationary, softmax denominator + relu folded into
    one tensor_scalar per 128-block, next batch's stage A drained as
    thunks into this batch's C/D/F emission.
"""
import numpy as np

import concourse.bass as bass
import concourse.mybir as mybir
import concourse.tile as tile
from concourse import bacc
from concourse.bass_utils import run_bass_kernel_spmd
from concourse.masks import make_identity

F32 = mybir.dt.float32
F32R = mybir.dt.float32r
BF16 = mybir.dt.bfloat16
AL = mybir.AluOpType
AF = mybir.ActivationFunctionType
AX = mybir.AxisListType

B_PER_CORE = 2
N = 1024
F = 64
T = 12
K = 3
FO = 64
NT = N // 128           # 8 n-tiles (128 rows each)
NH = 2                  # n processed in halves
HW = N // NH            # 512 free-dim per half
TQ = (T * F) // 128     # 6 (t,f)-chunks (each = 2 t-values x 64 f)


def _make_stage_a(nc, pools, cst, b, x_d):
    """Allocate stage-A tiles for batch b and return (thunks, tiles).

    Stage A is now just: DMA x in, cast/reorder to x' (m-part, bf16),
    and one XBAR DMA-transpose per m-tile into the (t,f)-partitioned
    xT chunks. Cut into per-m-tile thunks the caller interleaves into
    the PREVIOUS batch's C/D/F emission."""
    (stream, bigp, pe_pool, res_pool, psZ, psWC, psS, psT, dram_pool) = pools
    xprime = bigp.tile([128, NT, T, F], BF16, tag="xp", bufs=2, name="xprime")
    xTall = bigp.tile([128, TQ, N], BF16, tag="xT", bufs=1, name="xTall")
    thunks = []
    for mi in range(NT):
        MS = slice(mi * 128, (mi + 1) * 128)
        xnat = stream.tile([128, F, T], F32, tag="xnat", bufs=2, name="xnat")

        def t_dma(xnat=xnat, MS=MS):
            nc.gpsimd.dma_start(out=xnat[:], in_=x_d.ap()[b, MS])

        def t_xp(xnat=xnat, mi=mi):
            nc.vector.tensor_copy(xprime[:, mi],
                                  xnat[:].rearrange("p f t -> p t f"))

        def t_tr(mi=mi, MS=MS):
            # xTall[p=(dt,f), q, m] = xprime[m, t=2q+dt, f]
            nc.sync.dma_start_transpose(
                out=xTall[:, :, MS],
                in_=xprime[:, mi].rearrange("p t f -> p (t f)"))

        thunks += [t_dma, t_xp, t_tr]
    return thunks, (xprime, xTall)


def _emit_batch(nc, tc, pools, cst, b, x_d, out_d, a_tiles, next_thunks,
                carry):
    """Emit one batch's B/C/D/F pipeline, draining next batch's stage-A
    thunks at fixed points along the way. `carry` holds cheap deferred
    emissions (trailing res scales + out DMAs) from the previous half /
    batch whose XBAR-transpose inputs were still in flight; they are
    consumed during this batch's stage C so no engine FIFO ever
    head-of-line blocks on a DMA. Returns the new carry list."""
    (stream, bigp, pe_pool, res_pool, psZ, psWC, psS, psT, dram_pool) = pools
    xprime, xTall = a_tiles

    def drain(n=1):
        for _ in range(n):
            if next_thunks:
                next_thunks.pop(0)()

    # ---- Stage W: xw1T (64, N) and rhsBT (12, N) via accumulated PE
    # matmuls over the xT chunks with the combined W1/W3 stationary ----
    xw1T = stream.tile([F, N], BF16, tag="xw1T", bufs=2, name="xw1T")
    rhsBT = stream.tile([T, N], BF16, tag="rhsBT", bufs=2, name="rhsBT")
    for h in range(NH):
        HS = slice(h * HW, (h + 1) * HW)
        ps_w = psWC.tile([F + T, HW], F32, tag="wc", name="ps_w")
        for q in range(TQ):
            nc.tensor.matmul(ps_w[:], cst["wc"][:, q, :], xTall[:, q, HS],
                             start=(q == 0), stop=(q == TQ - 1))
        nc.scalar.copy(xw1T[:, HS], ps_w[0:F])
        nc.scalar.copy(rhsBT[:, HS], ps_w[F:F + T])

    # ---- Stage B: lhsT = W2^T @ xw1T  (12, N) ----
    # psum->sbuf copy on ACT: keeps stage C's LDWEIGHTS dependency off
    # the DVE FIFO at batch boundaries.
    lhsT_sb = stream.tile([T, N], BF16, tag="lhsT", bufs=1)
    for h in range(2):
        ps_l = psWC.tile([T, 512], F32, tag="wc")
        nc.tensor.matmul(ps_l[:], cst["w2r"][:], xw1T[:, h * 512:(h + 1) * 512],
                         start=True, stop=True)
        nc.scalar.copy(lhsT_sb[:, h * 512:(h + 1) * 512], ps_l[:])

    # ---- per n-half pipeline ----
    for nh in range(NH):
        HS = slice(nh * HW, (nh + 1) * HW)
        # Stage C: product + bs -> sigmoid -> P  (rows = m, cols = dest n)
        P_h = pe_pool.tile([128, NT, HW], BF16, tag="P", bufs=1)
        for mi in range(NT):
            MS = slice(mi * 128, (mi + 1) * 128)
            ps_p = psWC.tile([128, HW], F32, tag="wc")
            nc.tensor.matmul(ps_p[:], lhsT_sb[:, MS], rhsBT[:, HS],
                             start=True, stop=True)
            sgin = stream.tile([128, HW], BF16, tag="sgin", bufs=3)
            nc.vector.tensor_add(sgin[:], ps_p[:], cst["bs_sb"][:, mi, HS])
            nc.scalar.activation(P_h[:, mi], sgin[:], AF.Sigmoid)
            drain(1)
            if carry:
                carry.pop(0)()
        while carry:
            carry.pop(0)()
        # Stage D: S = VsT^T @ P ; E = exp(S).  A_k = cheb_k * E muls run on
        # DVE (k=0,1) and gpsimd (k=2) behind the PE; colsum matmuls are
        # deferred past the ii loop so the tensor queue never waits on exp.
        E_h = pe_pool.tile([128, NT, HW], BF16, tag="E", bufs=1)
        A_t = [pe_pool.tile([128, NT, HW], BF16, tag=f"A{k}", bufs=1,
                            name=f"A{k}") for k in range(K)]
        for ii in range(NT):
            ps_s = psS.tile([128, HW], F32, tag="s")
            for pi in range(NT):
                nc.tensor.matmul(ps_s[:], cst["vsT"][:, pi, ii * 128:(ii + 1) * 128],
                                 P_h[:, pi], start=(pi == 0), stop=(pi == NT - 1))
            nc.scalar.activation(E_h[:, ii], ps_s[:], AF.Exp)
            nc.vector.tensor_mul(A_t[0][:, ii], cst["chebb"][:, 0, ii, HS],
                                 E_h[:, ii])
            nc.vector.tensor_mul(A_t[1][:, ii], cst["chebb"][:, 1, ii, HS],
                                 E_h[:, ii])
            nc.gpsimd.tensor_mul(A_t[2][:, ii], cst["chebb"][:, 2, ii, HS],
                                 E_h[:, ii])
            drain(1)
        ps_cs = psS.tile([1, HW], F32, tag="s")
        for ii in range(NT):
            nc.tensor.matmul(ps_cs[:], cst["ones_b"][:], E_h[:, ii],
                             start=(ii == 0), stop=(ii == NT - 1))
        # softmax denominator reciprocal, scattered to partitions via DRAM
        cs_row = stream.tile([1, HW], F32, tag="cs", bufs=1)
        nc.vector.tensor_copy(cs_row[:], ps_cs[:])
        rc_d = dram_pool.tile([HW], F32, tag="rcd", name="rc_d")
        nc.sync.dma_start(out=rc_d.rearrange("(a b) -> a b", a=1), in_=cs_row[:])
        rc_sc = stream.tile([128, HW // 128], F32, tag="rcsc", bufs=2)
        nc.sync.dma_start(out=rc_sc[:], in_=rc_d.rearrange("(c p) -> p c", p=128))
        recip_sb = stream.tile([128, HW // 128], F32, tag="recip", bufs=2)
        nc.vector.reciprocal(recip_sb[:], rc_sc[:])
        # Stage F: z'_k = x'-chunk^T @ A_k ; Theta via block-diag; output
        # transpose on the DMA XBAR; relu/softmax-scale per 128-block.
        res_tiles = []
        for _c in range(HW // 128):
            res_c = res_pool.tile([128, FO, T], F32, tag=f"res{_c}", bufs=1,
                                  name=f"res{_c}")
            res_tiles.append(res_c)

        def _theta_mm(zs_tiles, q):
            """Theta block-diag + XBAR transpose for chunk q. Emitted one q
            behind the z' matmuls so the tensor queue never waits on the
            ACT psum->sbuf copies."""
            ps_o = psT.tile([128, HW], F32, tag="t", name="ps_o")
            for k in range(K):
                nc.tensor.matmul(ps_o[:], cst["thbd"][:, k, :], zs_tiles[k][:],
                                 start=(k == 0), stop=(k == K - 1))
            os_t = stream.tile([128, HW], BF16, tag="os", bufs=3)
            nc.scalar.copy(os_t[:], ps_o[:])
            tr = stream.tile([128, HW // 128, 128], BF16, tag="tr", bufs=3,
                             name="tr")
            # tr[p=n(within c), c, j=(dt,o)] = os_t[(dt,o), c*128+p]
            nc.sync.dma_start_transpose(out=tr[:], in_=os_t[:])
            return tr

        def _res_scale(tr, q, res_tiles=res_tiles, recip_sb=recip_sb):
            """relu/softmax-scale for chunk q. Runs two q behind the XBAR
            transpose so the DVE never head-of-line blocks on it."""
            for c in range(HW // 128):
                nc.vector.tensor_scalar(
                    out=res_tiles[c][:, :, 2 * q:2 * q + 2],
                    in0=tr[:, c].rearrange("p (dt o) -> p o dt", o=FO),
                    scalar1=recip_sb[:, c:c + 1],
                    scalar2=0.0,
                    op0=AL.mult,
                    op1=AL.max,
                )

        pend_mm = None
        pend_res = None
        for q in range(TQ):
            zs_tiles = []
            for k in range(K):
                ps_z = psZ.tile([128, HW], F32, tag="z")
                for mi in range(NT):
                    nc.tensor.matmul(ps_z[:], xprime[:, mi, 2 * q:2 * q + 2, :],
                                     A_t[k][:, mi],
                                     start=(mi == 0), stop=(mi == NT - 1))
                zs = stream.tile([128, HW], BF16, tag="zs", bufs=6)
                nc.scalar.copy(zs[:], ps_z[:])
                zs_tiles.append(zs)
            if pend_mm is not None:
                tr = _theta_mm(*pend_mm)
                if pend_res is not None:
                    _res_scale(*pend_res)
                pend_res = (tr, pend_mm[1])
            pend_mm = (zs_tiles, q)
            drain(2)
        tr = _theta_mm(*pend_mm)
        # Defer the two trailing res scales and the out DMAs into the next
        # stage-C loop (next half or next batch): by then their transposes
        # have long completed, so the DVE pops them without waiting.
        carry = [
            (lambda a=pend_res: _res_scale(*a)),
            (lambda a=(tr, pend_mm[1]): _res_scale(*a)),
        ]

        def _out_dma(c, nh=nh, res_tiles=res_tiles):
            nt_i = nh * (HW // 128) + c
            nc.sync.dma_start(
                out=out_d.ap()[b, nt_i * 128:(nt_i + 1) * 128],
                in_=res_tiles[c][:])

        carry += [(lambda c=c: _out_dma(c)) for c in range(HW // 128)]
    drain(len(next_thunks))
    return carry


def build_nc(repeat=1):
    nc = bacc.Bacc("TRN2", target_bir_lowering=False, debug=False, num_devices=8)
    x_d = nc.dram_tensor("x", [B_PER_CORE, N, F, T], F32, kind="ExternalInput")
    w1_d = nc.dram_tensor("W1", [T], F32, kind="ExternalInput")
    w2_d = nc.dram_tensor("W2", [F, T], F32, kind="ExternalInput")
    w3_d = nc.dram_tensor("W3", [F], F32, kind="ExternalInput")
    bs_d = nc.dram_tensor("bs", [N, N], F32, kind="ExternalInput")
    vs_d = nc.dram_tensor("Vs", [N, N], F32, kind="ExternalInput")
    cheb_d = nc.dram_tensor("cheb", [K, N, N], F32, kind="ExternalInput")
    th_d = nc.dram_tensor("Theta", [K, F, FO], F32, kind="ExternalInput")
    out_d = nc.dram_tensor("out", [B_PER_CORE, N, FO, T], F32,
                           kind="ExternalOutput")

    with tile.TileContext(nc) as tc:
        with (
            tc.tile_pool(name="consts", bufs=1) as consts,
            tc.tile_pool(name="stream", bufs=2) as stream,
            tc.tile_pool(name="bigp", bufs=1) as bigp,
            tc.tile_pool(name="pe", bufs=1) as pe_pool,
            tc.tile_pool(name="res", bufs=1) as res_pool,
            tc.tile_pool(name="dram", bufs=2, space="DRAM") as dram_pool,
            tc.tile_pool(name="psZ", bufs=2, space="PSUM") as psZ,
            tc.tile_pool(name="psWC", bufs=2, space="PSUM") as psWC,
            tc.tile_pool(name="psS", bufs=2, space="PSUM") as psS,
            tc.tile_pool(name="psT", bufs=2, space="PSUM") as psT,
        ):
            cst = {}
            identb = consts.tile([128, 128], BF16)
            make_identity(nc, identb[:])
            cst["identb"] = identb
            # ones vector (bf16) for the softmax column sums
            onesf = consts.tile([128, 1], F32)
            nc.vector.memset(onesf[:], 1.0)
            ones_b = consts.tile([128, 1], BF16)
            nc.vector.tensor_copy(ones_b[:], onesf[:])
            cst["ones_b"] = ones_b
            # ---- combined W1/W3 stationary WC [128, TQ, 76]:
            #   cols 0:64   -> xw1T rows:  WC[(dt,f), q, f'] = W1[2q+dt]*d(f==f')
            #   cols 64:76  -> rhsBT rows: WC[(dt,f), q, 64+t'] = W3[f]*d(t'==2q+dt)
            # d2[p, j] = d(p % 64 == j)  (two stacked 64-identities)
            d2 = consts.tile([128, F], BF16)
            nc.vector.tensor_copy(d2[0:F], identb[0:F, 0:F])
            nc.vector.tensor_copy(d2[F:128], identb[F:128, F:128])
            # W1 broadcast to all partitions: wcol[p, t] = W1[t]
            wcol = stream.tile([128, T], F32, tag="xnat", bufs=2, name="wcol")
            nc.gpsimd.dma_start(
                out=wcol[:],
                in_=bass.AP(tensor=w1_d, offset=0, ap=[[0, 128], [1, T]]))
            # w1col[p, q] = W1[2q + p//64]
            w1col = consts.tile([128, TQ], F32)
            for q in range(TQ):
                nc.vector.tensor_copy(w1col[0:F, q:q + 1],
                                      wcol[0:F, 2 * q:2 * q + 1])
                nc.vector.tensor_copy(w1col[F:128, q:q + 1],
                                      wcol[F:128, 2 * q + 1:2 * q + 2])
            # W3 broadcast, then w3col[p] = W3[p % 64] via d2 reduce
            wst2 = stream.tile([128, F], F32, tag="xnat", bufs=2, name="wst2")
            nc.gpsimd.dma_start(
                out=wst2[:],
                in_=bass.AP(tensor=w3_d, offset=0, ap=[[0, 128], [1, F]]))
            tmp64 = stream.tile([128, F], F32, tag="xnat", bufs=2, name="tmp64")
            nc.vector.tensor_mul(tmp64[:], d2[:], wst2[:])
            w3col = consts.tile([128, 1], F32)
            nc.vector.tensor_reduce(out=w3col[:], in_=tmp64[:], op=AL.add,
                                    axis=AX.X)
            wc = consts.tile([128, TQ, F + T], BF16)
            nc.vector.memset(wc[:], 0.0)
            for q in range(TQ):
                nc.vector.tensor_scalar(
                    out=wc[:, q, 0:F], in0=d2[:],
                    scalar1=w1col[:, q:q + 1], scalar2=0.0,
                    op0=AL.mult, op1=AL.add)
                nc.vector.tensor_copy(wc[0:F, q, F + 2 * q:F + 2 * q + 1],
                                      w3col[0:F])
                nc.vector.tensor_copy(wc[F:128, q, F + 2 * q + 1:F + 2 * q + 2],
                                      w3col[F:128])
            cst["wc"] = wc
            # W2 (f, t) bf16 stationary
            w2f = consts.tile([F, T], F32)
            nc.sync.dma_start(out=w2f[:], in_=w2_d.ap())
            w2r = consts.tile([F, T], BF16)
            nc.vector.tensor_copy(w2r[:], w2f[:])
            cst["w2r"] = w2r
            # block-diagonal Theta (128, K, 128), bf16 stationary
            thbd_f = consts.tile([128, K, 128], F32)
            nc.vector.memset(thbd_f[:], 0.0)
            for k in range(K):
                nc.sync.dma_start(out=thbd_f[0:F, k, 0:FO], in_=th_d.ap()[k])
                nc.sync.dma_start(out=thbd_f[F:128, k, FO:128], in_=th_d.ap()[k])
            thbd = consts.tile([128, K, 128], BF16)
            nc.vector.tensor_copy(thbd[:], thbd_f[:])
            cst["thbd"] = thbd
            # cheb cached in SBUF as bf16 (one-time cast) so the A_k muls
            # run at the 16-bit 2x DVE rate with no per-batch cheb DMA
            chebb = consts.tile([128, K, NT, N], BF16)
            for k in range(K):
                for mi in range(NT):
                    chst = stream.tile([128, N], F32, tag="xnat", bufs=2, name="chst")
                    nc.gpsimd.dma_start(
                        out=chst[:],
                        in_=cheb_d.ap()[k, mi * 128:(mi + 1) * 128])
                    nc.vector.tensor_copy(chebb[:, k, mi], chst[:])
            cst["chebb"] = chebb
            # bs cached in SBUF as bf16 (one-time cast)
            bs_sb = consts.tile([128, NT, N], BF16)
            for mi in range(NT):
                bst = stream.tile([128, N], F32, tag="xnat", bufs=2, name="bst")
                nc.gpsimd.dma_start(
                    out=bst[:], in_=bs_d.ap()[mi * 128:(mi + 1) * 128])
                nc.vector.tensor_copy(bs_sb[:, mi], bst[:])
            cst["bs_sb"] = bs_sb
            # VsT (m-partitioned Vs transpose), bf16 stationary
            vsT = consts.tile([128, NT, N], BF16)
            identf = consts.tile([128, 128], F32)
            make_identity(nc, identf[:])
            for pi in range(NT):
                for ii in range(NT):
                    vtmp = stream.tile([128, 128], F32, tag="xnat", bufs=2, name="vtmp")
                    nc.sync.dma_start(
                        out=vtmp[:],
                        in_=vs_d.ap()[ii * 128:(ii + 1) * 128,
                                      pi * 128:(pi + 1) * 128])
                    ps_v = psT.tile([128, 128], F32, tag="t")
                    nc.tensor.transpose(ps_v[:], vtmp[:], identf[:])
                    nc.vector.tensor_copy(vsT[:, pi, ii * 128:(ii + 1) * 128],
                                          ps_v[:])
            cst["vsT"] = vsT

            pools = (stream, bigp, pe_pool, res_pool, psZ, psWC, psS, psT, dram_pool)
            # software pipeline: batch b+1's stage A interleaves into batch
            # b's C/D/F emission; the first batch's stage A runs up front.
            a_thunks, a_tiles = _make_stage_a(nc, pools, cst, 0, x_d)
            for t in a_thunks:
                t()
            total = repeat * B_PER_CORE
            carry = []
            for i in range(total):
                b = i % B_PER_CORE
                if i + 1 < total:
                    n_thunks, n_tiles = _make_stage_a(nc, pools, cst,
                                                      (i + 1) % B_PER_CORE, x_d)
                else:
                    n_thunks, n_tiles = [], None
                carry = _emit_batch(nc, tc, pools, cst, b, x_d, out_d,
                                    a_tiles, n_thunks, carry)
                a_tiles = n_tiles
            for f in carry:
                f()
    nc.compile()
    return nc


_RUNNER_CACHE = {}


def _make_runner(repeat=1):
    """Build the Bass program once and wrap it in a persistent jitted
    shard_map executable so repeat calls skip recompile/reload."""
    import jax
    from jax.sharding import Mesh, PartitionSpec
    from jax.experimental.shard_map import shard_map
    from concourse import bass2jax, mybir as _mybir

    nc = build_nc(repeat)
    bass2jax.install_neuronx_cc_hook()

    part_name = nc.partition_id_tensor.name if nc.partition_id_tensor else None
    in_names = []
    out_names = []
    out_avals = []
    zero_outs = []
    for alloc in nc.m.functions[0].allocations:
        if not isinstance(_mybir.MemoryLocationSet, type) or not isinstance(
                alloc, _mybir.MemoryLocationSet):
            continue
        name = alloc.memorylocations[0].name
        if alloc.kind == "ExternalInput":
            if name != part_name:
                in_names.append(name)
        elif alloc.kind == "ExternalOutput":
            out_names.append(name)
            shape = tuple(alloc.tensor_shape)
            dtype = _mybir.dt.np(alloc.dtype)
            out_avals.append(jax.core.ShapedArray(shape, dtype))
            zero_outs.append(np.zeros(shape, dtype))
    n_params = len(in_names)
    all_names = in_names + out_names
    if part_name is not None:
        all_names = all_names + [part_name]

    def _body(*args):
        operands = list(args)
        if part_name is not None:
            operands.append(bass2jax.partition_id_tensor())
        outs = bass2jax._bass_exec_p.bind(
            *operands,
            out_avals=tuple(out_avals),
            in_names=tuple(all_names),
            out_names=tuple(out_names),
            lowering_input_output_aliases=(),
            sim_require_finite=False,
            sim_require_nnan=False,
            nc=nc,
        )
        return tuple(outs)

    n_cores = 8
    devices = jax.devices()[:n_cores]
    mesh = Mesh(np.asarray(devices), ("core",))
    in_specs = tuple(
        PartitionSpec("core") if name == "x" else PartitionSpec()
        for name in in_names
    ) + (PartitionSpec("core"),) * len(out_names)
    out_specs = (PartitionSpec("core"),) * len(out_names)
    sharded = jax.jit(
        shard_map(_body, mesh=mesh, in_specs=in_specs, out_specs=out_specs,
                  check_rep=False),
        keep_unused=True,
    )
    return nc, sharded, in_names, out_names, zero_outs, n_cores, mesh


def _get_runner(repeat=1):
    if repeat not in _RUNNER_CACHE:
        _RUNNER_CACHE[repeat] = _make_runner(repeat)
    return _RUNNER_CACHE[repeat]


def kernel(x, W1, W2, W3, bs, Vs, cheb, Theta, repeat=1):
    x = np.asarray(x, dtype=np.float32)
    full = {
        "W1": np.asarray(W1, dtype=np.float32),
        "W2": np.asarray(W2, dtype=np.float32),
        "W3": np.asarray(W3, dtype=np.float32),
        "bs": np.asarray(bs, dtype=np.float32).reshape(N, N),
        "Vs": np.asarray(Vs, dtype=np.float32),
        "cheb": np.asarray(cheb, dtype=np.float32),
        "Theta": np.asarray(Theta, dtype=np.float32),
    }
    nc, sharded, in_names, out_names, zero_outs, n_cores, mesh = _get_runner(repeat)
    ops = _staged_ops(x, full, in_names, zero_outs, n_cores)
    out_arrs = sharded(*ops)
    out = np.asarray(out_arrs[out_names.index("out")])
    return out.reshape(16, N, FO, T)


def _staged_ops(x, full, in_names, zero_outs, n_cores):
    ops = []
    for name in in_names:
        if name == "x":
            ops.append(np.ascontiguousarray(x.reshape(n_cores * B_PER_CORE, N, F, T)))
        else:
            ops.append(full[name])
    for z in zero_outs:
        ops.append(np.zeros((n_cores * z.shape[0], *z.shape[1:]), z.dtype))
    return ops


def _bench_setup(inputs, repeat):
    import jax
    from jax.sharding import NamedSharding, PartitionSpec
    x = np.asarray(inputs["x"], dtype=np.float32)
    full = {k: np.asarray(v, dtype=np.float32) for k, v in inputs.items() if k != "x"}
    full["bs"] = full["bs"].reshape(N, N)
    nc, sharded, in_names, out_names, zero_outs, n_cores, mesh = _get_runner(repeat)
    ops = _staged_ops(x, full, in_names, zero_outs, n_cores)
    sh_core = NamedSharding(mesh, PartitionSpec("core"))
    sh_rep = NamedSharding(mesh, PartitionSpec())
    shardings = [sh_core if name == "x" else sh_rep for name in in_names]
    shardings += [sh_core] * len(zero_outs)
    dev_ops = [jax.device_put(o, s_) for o, s_ in zip(ops, shardings)]
    jax.block_until_ready(sharded(*dev_ops))
    return sharded, dev_ops


def bench_pair(inputs, rep_a=1, rep_b=9, iters=20):
    """Interleaved device-resident timing of two repeat variants.
    Returns (best_a, best_b) seconds — interleaving cancels slow drift in the
    fixed dispatch overhead."""
    import time as _time
    import jax
    sh_a, ops_a = _bench_setup(inputs, rep_a)
    sh_b, ops_b = _bench_setup(inputs, rep_b)
    best_a = best_b = float("inf")
    for _ in range(iters):
        t0 = _time.time()
        jax.block_until_ready(sh_a(*ops_a))
        best_a = min(best_a, _time.time() - t0)
        t0 = _time.time()
        jax.block_until_ready(sh_b(*ops_b))
        best_b = min(best_b, _time.time() - t0)
    return best_a, best_b
